# revision 22
# baseline (speedup 1.0000x reference)
"""GAT (dense masked softmax attention) Bass kernel for 8 Trainium2 NeuronCores.

Row-parallel sharding: core c owns output rows [c*NB, (c+1)*NB). Each core
computes the full h = x @ W.T (replicated), then its row-block of the masked
attention softmax against all N nodes, in transposed layout (j on partitions,
own-rows on free dim), accumulating z @ [h | 1] in PSUM so the softmax
denominator falls out of the same matmuls.

Host prep (sharding only): x -> x.T, adjacency row-block -> additive mask
(0 at edges / -1e4 elsewhere) transposed to [N, NB], weight packing
(Wcat = [W.T | W.T@a2], b1 = W.T@a1 replicated across 128 columns).
"""

import contextlib
import ctypes
import sys
import types

import numpy as np
import ml_dtypes

import concourse.bacc as bacc
import concourse.mybir as mybir
import concourse.tile as tile
from concourse.masks import make_identity

P = 128
NEG_MASK = -1.0e4  # additive mask; prelu scales it by alpha=0.01 -> exp(~-100) ~ 0


def _install_ntff_hook():
    """Register the axon NTFF profile hook so run_bass_kernel_spmd(trace=True)
    can capture neuron-profile data (antenv.axon_hooks is absent here)."""
    if "antenv.axon_hooks" in sys.modules:
        return
    try:
        lib = ctypes.CDLL("/opt/axon/libaxon_pjrt.so")
        if not hasattr(lib, "axon_start_nrt_profile"):
            return
    except OSError:
        return
    lib.axon_start_nrt_profile.argtypes = [ctypes.POINTER(ctypes.c_int64), ctypes.c_size_t]
    lib.axon_start_nrt_profile.restype = ctypes.c_int64
    lib.axon_stop_nrt_profile.argtypes = [ctypes.c_char_p]
    lib.axon_stop_nrt_profile.restype = ctypes.c_int64

    @contextlib.contextmanager
    def _hook(output_dir, device_ids):
        import jax

        jax.devices()
        if device_ids:
            ids = (ctypes.c_int64 * len(device_ids))(*device_ids)
            rc = lib.axon_start_nrt_profile(ids, len(device_ids))
        else:
            rc = lib.axon_start_nrt_profile(None, 0)
        if rc != 0:
            raise RuntimeError(f"axon_start_nrt_profile rc={rc}")
        try:
            yield
        finally:
            n = lib.axon_stop_nrt_profile(str(output_dir).encode())
            print(f"ntff profile: {n} file(s) in {output_dir}", file=sys.stderr)

    mod = types.ModuleType("antenv.axon_hooks")
    mod.get_axon_ntff_profile_hook = lambda: _hook
    mod.set_axon_ntff_profile_hook = lambda h: None
    sys.modules["antenv.axon_hooks"] = mod


class GatConfig:
    def __init__(self, n=8192, d=512, h=256, c=16, n_cores=8, s_f32=True):
        assert n % (n_cores * P) == 0 and d % P == 0 and h % P == 0
        self.n, self.d, self.h, self.c, self.n_cores = n, d, h, c, n_cores
        self.nb = n // n_cores          # own rows per core
        self.nch = n // P               # j-chunks (also m-tiles of h)
        self.ndc = d // P               # feature chunks
        self.nit = self.nb // P         # own i-tiles
        self.dt_x = mybir.dt.bfloat16   # x / weights path
        self.dt_z = mybir.dt.bfloat16   # post-exp z and h matmul operands
        self.dt_s = mybir.dt.float32 if s_f32 else mybir.dt.bfloat16  # pre-exp path

    def key(self):
        return (self.n, self.d, self.h, self.c, self.n_cores, self.dt_s)


def build_gat(cfg: GatConfig):
    """Build + compile the SPMD Bass program (identical on all cores)."""
    nc = bacc.Bacc("TRN2", target_bir_lowering=False, debug=False,
                   enable_asserts=False, num_devices=cfg.n_cores)
    N, D, H, C = cfg.n, cfg.d, cfg.h, cfg.c
    NB, NCH, NDC, NIT = cfg.nb, cfg.nch, cfg.ndc, cfg.nit
    f32 = mybir.dt.float32
    bf16 = mybir.dt.bfloat16

    xT = nc.dram_tensor("xT", [D, N], cfg.dt_x, kind="ExternalInput").ap()
    xTown = nc.dram_tensor("xTown", [D, NB], cfg.dt_x, kind="ExternalInput").ap()
    wcat = nc.dram_tensor("wcat", [D, H + 1], cfg.dt_x, kind="ExternalInput").ap()
    b1rep = nc.dram_tensor("b1rep", [D, P], cfg.dt_x, kind="ExternalInput").ap()
    maskT = nc.dram_tensor("maskT", [N, NB], bf16, kind="ExternalInput").ap()
    fcwT = nc.dram_tensor("fcwT", [H, C], f32, kind="ExternalInput").ap()
    fcb = nc.dram_tensor("fcb", [C, 1], f32, kind="ExternalInput").ap()
    logitsT = nc.dram_tensor("logitsT", [C, NB], f32, kind="ExternalOutput").ap()

    AF = mybir.ActivationFunctionType
    OP = mybir.AluOpType

    with tile.TileContext(nc) as tc:
        with (
            tc.tile_pool(name="persist", bufs=1) as pp,
            tc.tile_pool(name="mwork", bufs=6) as mwp,
            tc.tile_pool(name="swork", bufs=3) as swp,
            tc.tile_pool(name="zwork", bufs=3) as zwp,
            tc.tile_pool(name="tail", bufs=2) as tp,
        ):
            # ---------------- resident inputs ----------------
            xo_sb = []
            for dd in range(NDC):
                t = pp.tile([P, NB], cfg.dt_x, tag=f"xo{dd}")
                nc.sync.dma_start(t[:], xTown[dd * P:(dd + 1) * P, :])
                xo_sb.append(t)
            wcat_sb = []
            for dd in range(NDC):
                t = pp.tile([P, H + 1], cfg.dt_x, tag=f"wc{dd}")
                nc.sync.dma_start(t[:], wcat[dd * P:(dd + 1) * P, :])
                wcat_sb.append(t)
            b1_sb = []
            for dd in range(NDC):
                t = pp.tile([P, P], cfg.dt_x, tag=f"b1{dd}")
                nc.sync.dma_start(t[:], b1rep[dd * P:(dd + 1) * P, :])
                b1_sb.append(t)
            fcw_sb = []
            for hh in range(H // P):
                t = pp.tile([P, C], f32, tag=f"fcw{hh}")
                nc.sync.dma_start(t[:], fcwT[hh * P:(hh + 1) * P, :])
                fcw_sb.append(t)
            fcb_sb = pp.tile([C, 1], f32, tag="fcb")
            nc.sync.dma_start(fcb_sb[:], fcb[:])
            ident = pp.tile([P, P], f32, tag="ident")
            make_identity(nc, ident[:])

            f1b_sb = pp.tile([P, NB], cfg.dt_s, tag="f1b")
            f2_sb = pp.tile([P, NCH], f32, tag="f2")
            h_sb = [pp.tile([P, H], cfg.dt_z, tag=f"h{m}", name=f"h{m}")
                    for m in range(NCH)]
            onecol = pp.tile([P, 1], cfg.dt_z, tag="onecol")
            nc.gpsimd.memset(onecol[:], 1.0)
            one1 = pp.tile([1, 1], f32, tag="one1")
            nc.gpsimd.memset(one1[:], 1.0)

            MB = NIT                     # m-tiles per xT column block
            GROUP = 2 if NCH % 2 == 0 else 1  # chunks per wide ACT op
            NACC = (NIT + 1) // 2        # packed accumulator tiles (2 i-tiles/bank)
            nq = (NB + 511) // 512       # 512-wide column groups of NB
            xtb = {}

            # Accumulators (4 banks) + denominator rows (2 banks) coexist with
            # the h-pipeline psum (2 banks): merged single-pass emission, so
            # the first sm/ACT work is not queued behind all of phase 1 on the
            # in-order DVE/sync engines.
            with tc.tile_pool(name="acc", bufs=1, space="PSUM") as accp:
                acc = [accp.tile([P, 2 * H], f32, tag=f"acc{q}", name=f"acc{q}")
                       for q in range(NACC)]
                dn = [accp.tile([1, min(512, NB - q * 512)], f32, tag=f"dn{q}",
                                name=f"dn{q}")
                      for q in range(nq)]

                with tc.tile_pool(name="ps1", bufs=2, space="PSUM") as ps1:
                    # f1b[p, i] = sum_d b1rep[d, p] * xTown[d, i] (same for all
                    # p); borrows acc tiles as transient psum before the
                    # accumulation groups start.
                    for q in range(nq):
                        w = min(512, NB - q * 512)
                        fps = acc[q % NACC][:, 0:w]
                        for dd in range(NDC):
                            nc.tensor.matmul(fps, b1_sb[dd][:],
                                             xo_sb[dd][:, q * 512:q * 512 + w],
                                             start=(dd == 0), stop=(dd == NDC - 1))
                        nc.vector.tensor_copy(f1b_sb[:, q * 512:q * 512 + w], fps)

                    def quad_close(qc):
                        """Emit Prelu+Exp+accumulation for quad qc (chunks
                        qc*GROUP .. qc*GROUP+GROUP-1). Called one quad behind
                        the front-end emission so PE's h-matmuls for the next
                        quad sit BEFORE this quad's accumulation burst in
                        program order (breaks the quad-serializing cycle
                        through f2/hps)."""
                        smq = smq_of[qc]
                        zq = zwp.tile([P, GROUP * NB], cfg.dt_z, tag="z",
                                      name=f"z{qc}")
                        if qc in dve_lrelu_pairs:
                            # smq already leaky-relu'd on the vector engine
                            nc.scalar.activation(zq[:], smq[:], AF.Exp)
                        else:
                            uq = swp.tile([P, GROUP * NB], cfg.dt_s, tag="u",
                                          name=f"u{qc}")
                            nc.scalar.activation(uq[:], smq[:], AF.Prelu,
                                                 bias=0.0, scale=1.0, alpha=0.01)
                            nc.scalar.activation(zq[:], uq[:], AF.Exp)
                        for gq in range(GROUP):
                            gch = qc * GROUP + gq
                            z = zq[:, gq * NB:(gq + 1) * NB]
                            for it in range(NIT):
                                # Two accumulation groups share each psum
                                # bank; start=True clears the WHOLE bank, so
                                # only the even group (emitted first at ch==0)
                                # may clear. The odd group's first matmul
                                # writes (not accumulates): has_written is 0.
                                nc.tensor.matmul(
                                    acc[it // 2][:, (it % 2) * H:(it % 2) * H + H],
                                    z[:, it * P:(it + 1) * P], h_sb[gch][:],
                                    start=(gch == 0 and it % 2 == 0),
                                    stop=(gch == NCH - 1),
                                    skip_group_check=True)
                            for q in range(nq):
                                w = min(512, NB - q * 512)
                                nc.tensor.matmul(dn[q][:], onecol[:],
                                                 z[:, q * 512:q * 512 + w],
                                                 start=(gch == 0),
                                                 stop=(gch == NCH - 1))

                    npairs = NCH // GROUP
                    dve_lrelu_pairs = {p for p in range(npairs)
                                       if npairs >= 16 and p % 6 == 3}
                    smq_of = {}
                    for ch in range(NCH):
                        cb, mi = divmod(ch, MB)
                        if mi == 0:
                            for dd in range(NDC):
                                t = mwp.tile([P, MB * P], cfg.dt_x,
                                             tag=f"xtb{dd}", bufs=2,
                                             name=f"xtb{dd}_{cb}")
                                nc.sync.dma_start(
                                    t[:], xT[dd * P:(dd + 1) * P,
                                             cb * MB * P:(cb + 1) * MB * P])
                                xtb[dd, cb] = t
                        mk = mwp.tile([P, NB], bf16, tag="mask")
                        nc.sync.dma_start(mk[:], maskT[ch * P:(ch + 1) * P, :])

                        # h m-tile ch: [P, H+1] = sum_d xT_d[:, ch].T @ wcat_d
                        hps = ps1.tile([P, H + 1], f32, tag="hps")
                        for dd in range(NDC):
                            nc.tensor.matmul(hps[:],
                                             xtb[dd, cb][:, mi * P:(mi + 1) * P],
                                             wcat_sb[dd][:],
                                             start=(dd == 0), stop=(dd == NDC - 1))

                        # front end: smq slice = (f1 + f2[ch]) + mask; the
                        # Prelu is bias-free so GROUP chunks share one wide
                        # ACT op (amortizes the ~224-cycle ACT overhead).
                        qi = ch % GROUP
                        if qi == 0:
                            smq_of[ch // GROUP] = swp.tile(
                                [P, GROUP * NB], cfg.dt_s, tag="sm",
                                name=f"sm{ch // GROUP}")
                        smq = smq_of[ch // GROUP]
                        nc.vector.tensor_copy(f2_sb[:, ch:ch + 1], hps[:, H:H + 1])
                        sl = smq[:, qi * NB:(qi + 1) * NB]
                        # s = f1 + f2 (f2 column read straight from psum)
                        nc.vector.tensor_scalar(out=sl, in0=f1b_sb[:],
                                                scalar1=hps[:, H:H + 1],
                                                scalar2=None, op0=OP.add)
                        # mask add: one chunk of each pair on the (otherwise
                        # idle) gpsimd engine, the other on the vector engine
                        if qi == 0 and NCH >= 32:
                            nc.gpsimd.tensor_tensor(out=sl, in0=sl, in1=mk[:],
                                                    op=OP.add)
                        else:
                            nc.vector.tensor_tensor(out=sl, in0=sl, in1=mk[:],
                                                    op=OP.add)
                        if (ch // GROUP) in dve_lrelu_pairs:
                            # lrelu(x) = max(x, 0.01x) on DVE instead of ACT
                            p01 = mwp.tile([P, NB], cfg.dt_s, tag="p01", bufs=3)
                            nc.vector.tensor_scalar_mul(p01[:], sl, 0.01)
                            nc.vector.tensor_tensor(out=sl, in0=sl, in1=p01[:],
                                                    op=OP.max)
                        nc.vector.tensor_copy(h_sb[ch][:], hps[:, 0:H])
                        if qi == GROUP - 1 and ch // GROUP >= 1:
                            quad_close(ch // GROUP - 1)
                    quad_close(NCH // GROUP - 1)

                # ---------------- tail A: normalize + ELU ----------------
                dnrow = pp.tile([1, NB], f32, tag="dnrow")
                for q in range(nq):
                    w = min(512, NB - q * 512)
                    nc.vector.tensor_copy(dnrow[0:1, q * 512:q * 512 + w], dn[q][:])
                oe = []
                with tc.tile_pool(name="psD", bufs=2, space="PSUM") as psD:
                    for it in range(NIT):
                        dnc = psD.tile([P, 1], f32, tag="dnc")
                        nc.tensor.matmul(dnc[:],
                                         dnrow[0:1, it * P:(it + 1) * P],
                                         one1[:], start=True, stop=True)
                        rec = tp.tile([P, 1], f32, tag="rec")
                        nc.vector.reciprocal(rec[:], dnc[:])
                        a = acc[it // 2][:, (it % 2) * H:(it % 2) * H + H]
                        neg = tp.tile([P, H], f32, tag="neg")
                        nc.vector.tensor_scalar(out=neg[:], in0=a, scalar1=rec[:],
                                                scalar2=0.0, op0=OP.mult,
                                                op1=OP.min)
                        pos = tp.tile([P, H], f32, tag="pos")
                        nc.vector.tensor_scalar(out=pos[:], in0=a, scalar1=rec[:],
                                                scalar2=0.0, op0=OP.mult,
                                                op1=OP.max)
                        ex = tp.tile([P, H], f32, tag="ex")
                        nc.scalar.activation(ex[:], neg[:], AF.Exp)
                        o = pp.tile([P, H], f32, tag=f"oe{it}", name=f"oe{it}")
                        nc.vector.scalar_tensor_tensor(out=o[:], in0=ex[:],
                                                       scalar=-1.0, in1=pos[:],
                                                       op0=OP.add, op1=OP.add)
                        oe.append(o)

            # ---------------- tail B: logitsT = fc_w @ oe.T + b ----------------
            logT = pp.tile([C, NB], f32, tag="logT")
            with tc.tile_pool(name="ps3", bufs=2, space="PSUM") as ps3:
                for it in range(NIT):
                    oT = []
                    for hh in range(H // P):
                        tps = ps3.tile([P, P], f32, tag="tps")
                        nc.tensor.transpose(tps[:], oe[it][:, hh * P:(hh + 1) * P],
                                            ident[:])
                        ot = tp.tile([P, P], f32, tag="ot")
                        nc.vector.tensor_copy(ot[:], tps[:])
                        oT.append(ot)
                    lps = ps3.tile([C, P], f32, tag="lps")
                    for hh in range(H // P):
                        nc.tensor.matmul(lps[:], fcw_sb[hh][:], oT[hh][:],
                                         start=(hh == 0), stop=(hh == H // P - 1))
                    nc.vector.tensor_scalar(out=logT[:, it * P:(it + 1) * P],
                                            in0=lps[:], scalar1=fcb_sb[:],
                                            scalar2=None, op0=OP.add)
            nc.sync.dma_start(logitsT[:], logT[:])

    nc.compile()
    return nc


# ---------------------------------------------------------------------------
# Host-side prep + execution
# ---------------------------------------------------------------------------

_CACHE = {}


def _get_nc(cfg: GatConfig):
    k = cfg.key()
    if k not in _CACHE:
        _CACHE[k] = build_gat(cfg)
    return _CACHE[k]


def prep_inputs(cfg, x, edge_index, W, a1, a2, fc_w, fc_b):
    """Shard + pack host inputs -> list of per-core in_maps."""
    bf = ml_dtypes.bfloat16
    N, NB = cfg.n, cfg.nb
    x = np.asarray(x, np.float32)
    W = np.asarray(W, np.float32)
    xT = np.ascontiguousarray(x.T).astype(bf)                    # [D, N]
    b1 = (W.T @ np.asarray(a1, np.float32)).astype(np.float32)   # [D, 1]
    b2 = (W.T @ np.asarray(a2, np.float32)).astype(np.float32)
    wcat = np.concatenate([W.T, b2], axis=1).astype(bf)          # [D, H+1]
    b1rep = np.repeat(b1, P, axis=1).astype(bf)                  # [D, P]
    fcwT = np.ascontiguousarray(np.asarray(fc_w, np.float32).T)  # [H, C]
    fcb = np.asarray(fc_b, np.float32).reshape(-1, 1)            # [C, 1]

    src = np.asarray(edge_index[0])
    dst = np.asarray(edge_index[1])
    in_maps = []
    for c in range(cfg.n_cores):
        lo = c * NB
        maskT = np.full((N, NB), NEG_MASK, np.float32)
        sel = (src >= lo) & (src < lo + NB)
        maskT[dst[sel], src[sel] - lo] = 0.0
        diag = np.arange(NB)
        maskT[lo + diag, diag] = 0.0
        in_maps.append({
            "xT": xT,
            "xTown": np.ascontiguousarray(xT[:, lo:lo + NB]),
            "wcat": wcat,
            "b1rep": b1rep,
            "maskT": maskT.astype(bf),
            "fcwT": fcwT,
            "fcb": fcb,
        })
    return in_maps


def run(cfg, inputs, trace=False):
    """Compile (cached), run on the 8 cores, return (logits, BassKernelResults)."""
    _install_ntff_hook()
    from concourse.bass_utils import run_bass_kernel_spmd

    nc = _get_nc(cfg)
    in_maps = prep_inputs(cfg, **inputs)
    res = run_bass_kernel_spmd(nc, in_maps, core_ids=list(range(cfg.n_cores)),
                               trace=trace)
    logits = np.concatenate(
        [np.asarray(res.results[c]["logitsT"], np.float32).T
         for c in range(cfg.n_cores)], axis=0)
    return logits, res


def kernel(x, edge_index, W, a1, a2, fc_w, fc_b):
    cfg = GatConfig(n=x.shape[0], d=x.shape[1], h=W.shape[0], c=fc_w.shape[0])
    logits, _ = run(cfg, dict(x=x, edge_index=edge_index, W=W, a1=a1, a2=a2,
                              fc_w=fc_w, fc_b=fc_b))
    return logits


# revision 24
# speedup vs baseline: 1.0256x; 1.0256x over previous
"""GAT (dense masked softmax attention) Bass kernel for 8 Trainium2 NeuronCores.

Row-parallel sharding: core c owns output rows [c*NB, (c+1)*NB). Each core
computes the full h = x @ W.T (replicated), then its row-block of the masked
attention softmax against all N nodes, in transposed layout (j on partitions,
own-rows on free dim), accumulating z @ [h | 1] in PSUM so the softmax
denominator falls out of the same matmuls.

Host prep (sharding only): x -> x.T, adjacency row-block -> additive mask
(0 at edges / -1e4 elsewhere) transposed to [N, NB], weight packing
(Wcat = [W.T | W.T@a2], b1 = W.T@a1 replicated across 128 columns).
"""

import contextlib
import ctypes
import sys
import types

import numpy as np
import ml_dtypes

import concourse.bacc as bacc
import concourse.mybir as mybir
import concourse.tile as tile
from concourse.masks import make_identity

P = 128
NEG_MASK = -1.0e4  # additive mask; prelu scales it by alpha=0.01 -> exp(~-100) ~ 0


def _install_ntff_hook():
    """Register the axon NTFF profile hook so run_bass_kernel_spmd(trace=True)
    can capture neuron-profile data (antenv.axon_hooks is absent here)."""
    if "antenv.axon_hooks" in sys.modules:
        return
    try:
        lib = ctypes.CDLL("/opt/axon/libaxon_pjrt.so")
        if not hasattr(lib, "axon_start_nrt_profile"):
            return
    except OSError:
        return
    lib.axon_start_nrt_profile.argtypes = [ctypes.POINTER(ctypes.c_int64), ctypes.c_size_t]
    lib.axon_start_nrt_profile.restype = ctypes.c_int64
    lib.axon_stop_nrt_profile.argtypes = [ctypes.c_char_p]
    lib.axon_stop_nrt_profile.restype = ctypes.c_int64

    @contextlib.contextmanager
    def _hook(output_dir, device_ids):
        import jax

        jax.devices()
        if device_ids:
            ids = (ctypes.c_int64 * len(device_ids))(*device_ids)
            rc = lib.axon_start_nrt_profile(ids, len(device_ids))
        else:
            rc = lib.axon_start_nrt_profile(None, 0)
        if rc != 0:
            raise RuntimeError(f"axon_start_nrt_profile rc={rc}")
        try:
            yield
        finally:
            n = lib.axon_stop_nrt_profile(str(output_dir).encode())
            print(f"ntff profile: {n} file(s) in {output_dir}", file=sys.stderr)

    mod = types.ModuleType("antenv.axon_hooks")
    mod.get_axon_ntff_profile_hook = lambda: _hook
    mod.set_axon_ntff_profile_hook = lambda h: None
    sys.modules["antenv.axon_hooks"] = mod


class GatConfig:
    def __init__(self, n=8192, d=512, h=256, c=16, n_cores=8, s_f32=True):
        assert n % (n_cores * P) == 0 and d % P == 0 and h % P == 0
        self.n, self.d, self.h, self.c, self.n_cores = n, d, h, c, n_cores
        self.nb = n // n_cores          # own rows per core
        self.nch = n // P               # j-chunks (also m-tiles of h)
        self.ndc = d // P               # feature chunks
        self.nit = self.nb // P         # own i-tiles
        self.dt_x = mybir.dt.bfloat16   # x / weights path
        self.dt_z = mybir.dt.bfloat16   # post-exp z and h matmul operands
        self.dt_s = mybir.dt.float32 if s_f32 else mybir.dt.bfloat16  # pre-exp path

    def key(self):
        return (self.n, self.d, self.h, self.c, self.n_cores, self.dt_s)


def build_gat(cfg: GatConfig):
    """Build + compile the SPMD Bass program (identical on all cores)."""
    nc = bacc.Bacc("TRN2", target_bir_lowering=False, debug=False,
                   enable_asserts=False, num_devices=cfg.n_cores)
    N, D, H, C = cfg.n, cfg.d, cfg.h, cfg.c
    NB, NCH, NDC, NIT = cfg.nb, cfg.nch, cfg.ndc, cfg.nit
    f32 = mybir.dt.float32
    bf16 = mybir.dt.bfloat16

    xT = nc.dram_tensor("xT", [D, N], cfg.dt_x, kind="ExternalInput").ap()
    xTown = nc.dram_tensor("xTown", [D, NB], cfg.dt_x, kind="ExternalInput").ap()
    wcat = nc.dram_tensor("wcat", [D, H + 1], cfg.dt_x, kind="ExternalInput").ap()
    b1rep = nc.dram_tensor("b1rep", [D, P], cfg.dt_x, kind="ExternalInput").ap()
    maskT = nc.dram_tensor("maskT", [N, NB], bf16, kind="ExternalInput").ap()
    fcwT = nc.dram_tensor("fcwT", [H, C], f32, kind="ExternalInput").ap()
    fcb = nc.dram_tensor("fcb", [C, 1], f32, kind="ExternalInput").ap()
    logitsT = nc.dram_tensor("logitsT", [C, NB], f32, kind="ExternalOutput").ap()

    AF = mybir.ActivationFunctionType
    OP = mybir.AluOpType

    with tile.TileContext(nc) as tc:
        with (
            tc.tile_pool(name="persist", bufs=1) as pp,
            tc.tile_pool(name="mwork", bufs=6) as mwp,
            tc.tile_pool(name="swork", bufs=3) as swp,
            tc.tile_pool(name="zwork", bufs=3) as zwp,
            tc.tile_pool(name="tail", bufs=2) as tp,
        ):
            # ---------------- resident inputs ----------------
            xo_sb = []
            for dd in range(NDC):
                t = pp.tile([P, NB], cfg.dt_x, tag=f"xo{dd}")
                nc.sync.dma_start(t[:], xTown[dd * P:(dd + 1) * P, :])
                xo_sb.append(t)
            wcat_sb = []
            for dd in range(NDC):
                t = pp.tile([P, H + 1], cfg.dt_x, tag=f"wc{dd}")
                nc.sync.dma_start(t[:], wcat[dd * P:(dd + 1) * P, :])
                wcat_sb.append(t)
            b1_sb = []
            for dd in range(NDC):
                t = pp.tile([P, P], cfg.dt_x, tag=f"b1{dd}")
                nc.sync.dma_start(t[:], b1rep[dd * P:(dd + 1) * P, :])
                b1_sb.append(t)
            fcw_sb = []
            for hh in range(H // P):
                t = pp.tile([P, C], f32, tag=f"fcw{hh}")
                nc.sync.dma_start(t[:], fcwT[hh * P:(hh + 1) * P, :])
                fcw_sb.append(t)
            fcb_sb = pp.tile([C, 1], f32, tag="fcb")
            nc.sync.dma_start(fcb_sb[:], fcb[:])
            ident = pp.tile([P, P], f32, tag="ident")
            make_identity(nc, ident[:])

            f1b_sb = pp.tile([P, NB], cfg.dt_s, tag="f1b")
            f2_sb = pp.tile([P, NCH], f32, tag="f2")
            h_sb = [pp.tile([P, H], cfg.dt_z, tag=f"h{m}", name=f"h{m}")
                    for m in range(NCH)]
            onecol = pp.tile([P, 1], cfg.dt_z, tag="onecol")
            nc.gpsimd.memset(onecol[:], 1.0)
            one1 = pp.tile([1, 1], f32, tag="one1")
            nc.gpsimd.memset(one1[:], 1.0)

            MB = NIT                     # m-tiles per xT column block
            GROUP = 2 if NCH % 2 == 0 else 1  # chunks per wide ACT op
            NACC = (NIT + 1) // 2        # packed accumulator tiles (2 i-tiles/bank)
            nq = (NB + 511) // 512       # 512-wide column groups of NB
            xtb = {}

            # Accumulators (4 banks) + denominator rows (2 banks) coexist with
            # the h-pipeline psum (2 banks): merged single-pass emission, so
            # the first sm/ACT work is not queued behind all of phase 1 on the
            # in-order DVE/sync engines.
            with tc.tile_pool(name="acc", bufs=1, space="PSUM") as accp:
                acc = [accp.tile([P, 2 * H], f32, tag=f"acc{q}", name=f"acc{q}")
                       for q in range(NACC)]
                dn = [accp.tile([1, min(512, NB - q * 512)], f32, tag=f"dn{q}",
                                name=f"dn{q}")
                      for q in range(nq)]

                with tc.tile_pool(name="ps1", bufs=2, space="PSUM") as ps1:
                    # f1b[p, i] = sum_d b1rep[d, p] * xTown[d, i] (same for all
                    # p); borrows acc tiles as transient psum before the
                    # accumulation groups start.
                    for q in range(nq):
                        w = min(512, NB - q * 512)
                        fps = acc[q % NACC][:, 0:w]
                        for dd in range(NDC):
                            nc.tensor.matmul(fps, b1_sb[dd][:],
                                             xo_sb[dd][:, q * 512:q * 512 + w],
                                             start=(dd == 0), stop=(dd == NDC - 1))
                        nc.vector.tensor_copy(f1b_sb[:, q * 512:q * 512 + w], fps)

                    # Per-chunk front end, multiplicative mask applied
                    # AFTER exp: ACT's Prelu reads the constant f1 broadcast
                    # tile directly (bias = f2 column), so ACT is gated only
                    # by a tiny f2 copy -- not by mask DMA or DVE grid ops.
                    for ch in range(NCH):
                        cb, mi = divmod(ch, MB)
                        if mi == 0:
                            for dd in range(NDC):
                                t = mwp.tile([P, MB * P], cfg.dt_x,
                                             tag=f"xtb{dd}", bufs=2,
                                             name=f"xtb{dd}_{cb}")
                                nc.sync.dma_start(
                                    t[:], xT[dd * P:(dd + 1) * P,
                                             cb * MB * P:(cb + 1) * MB * P])
                                xtb[dd, cb] = t
                        mk = mwp.tile([P, NB], bf16, tag="mask")
                        nc.sync.dma_start(mk[:], maskT[ch * P:(ch + 1) * P, :])

                        # h m-tile ch: [P, H+1] = sum_d xT_d[:, ch].T @ wcat_d
                        hps = ps1.tile([P, H + 1], f32, tag="hps")
                        for dd in range(NDC):
                            nc.tensor.matmul(hps[:],
                                             xtb[dd, cb][:, mi * P:(mi + 1) * P],
                                             wcat_sb[dd][:],
                                             start=(dd == 0), stop=(dd == NDC - 1))

                        u = swp.tile([P, NB], cfg.dt_s, tag="u")
                        if (ch % 3 == 1) and NCH >= 32:
                            # leaky-relu on DVE to relieve the scalar engine:
                            # u = max(s, 0.01*s), s = f1 + f2
                            sm = swp.tile([P, NB], cfg.dt_s, tag="sm", bufs=2)
                            nc.vector.tensor_scalar(out=sm[:], in0=f1b_sb[:],
                                                    scalar1=hps[:, H:H + 1],
                                                    scalar2=None, op0=OP.add)
                            p01 = swp.tile([P, NB], cfg.dt_s, tag="p01", bufs=2)
                            nc.vector.tensor_scalar_mul(p01[:], sm[:], 0.01)
                            nc.vector.tensor_tensor(out=u[:], in0=sm[:],
                                                    in1=p01[:], op=OP.max)
                        else:
                            f2c = tp.tile([P, 1], f32, tag="f2c", bufs=4)
                            nc.vector.tensor_copy(f2c[:], hps[:, H:H + 1])
                            nc.scalar.activation(u[:], f1b_sb[:], AF.Prelu,
                                                 bias=f2c[:], scale=1.0,
                                                 alpha=0.01)
                        ez = swp.tile([P, NB], cfg.dt_s, tag="ez")
                        nc.scalar.activation(ez[:], u[:], AF.Exp)
                        z = zwp.tile([P, NB], cfg.dt_z, tag="z", bufs=4)
                        if ch % 2 == 0 and NCH >= 32:
                            nc.gpsimd.tensor_tensor(out=z[:], in0=ez[:],
                                                    in1=mk[:], op=OP.mult)
                        else:
                            nc.vector.tensor_tensor(out=z[:], in0=ez[:],
                                                    in1=mk[:], op=OP.mult)
                        nc.vector.tensor_copy(h_sb[ch][:], hps[:, 0:H])

                        # inline accumulation: acc += z_it.T @ h ; dn += 1.T @ z
                        for it in range(NIT):
                            # Two accumulation groups share each psum bank;
                            # start=True clears the WHOLE bank, so only the
                            # even group (emitted first at ch==0) may clear.
                            # The odd group's first matmul writes (not
                            # accumulates): has_written is 0 there.
                            nc.tensor.matmul(
                                acc[it // 2][:, (it % 2) * H:(it % 2) * H + H],
                                z[:, it * P:(it + 1) * P], h_sb[ch][:],
                                start=(ch == 0 and it % 2 == 0),
                                stop=(ch == NCH - 1),
                                skip_group_check=True)
                        for q in range(nq):
                            w = min(512, NB - q * 512)
                            nc.tensor.matmul(dn[q][:], onecol[:],
                                             z[:, q * 512:q * 512 + w],
                                             start=(ch == 0),
                                             stop=(ch == NCH - 1))

                # ---------------- tail A: normalize + ELU ----------------
                dnrow = pp.tile([1, NB], f32, tag="dnrow")
                for q in range(nq):
                    w = min(512, NB - q * 512)
                    nc.vector.tensor_copy(dnrow[0:1, q * 512:q * 512 + w], dn[q][:])
                oe = []
                with tc.tile_pool(name="psD", bufs=2, space="PSUM") as psD:
                    for it in range(NIT):
                        dnc = psD.tile([P, 1], f32, tag="dnc")
                        nc.tensor.matmul(dnc[:],
                                         dnrow[0:1, it * P:(it + 1) * P],
                                         one1[:], start=True, stop=True)
                        rec = tp.tile([P, 1], f32, tag="rec")
                        nc.vector.reciprocal(rec[:], dnc[:])
                        recn = tp.tile([P, 1], f32, tag="recn")
                        nc.vector.tensor_scalar_mul(recn[:], rec[:], -1.0)
                        a = acc[it // 2][:, (it % 2) * H:(it % 2) * H + H]
                        # elu(on) = relu(on) + exp(min(on, 0)) - 1, with
                        # on = acc/denom done via per-partition scale APs
                        pos = tp.tile([P, H], f32, tag="pos")
                        nc.scalar.activation(pos[:], a, AF.Relu, scale=rec[:])
                        r2 = tp.tile([P, H], f32, tag="r2")
                        nc.scalar.activation(r2[:], a, AF.Relu, scale=recn[:])
                        ex = tp.tile([P, H], f32, tag="ex")
                        nc.scalar.activation(ex[:], r2[:], AF.Exp, scale=-1.0)
                        o = pp.tile([P, H], f32, tag=f"oe{it}", name=f"oe{it}")
                        nc.vector.scalar_tensor_tensor(out=o[:], in0=ex[:],
                                                       scalar=-1.0, in1=pos[:],
                                                       op0=OP.add, op1=OP.add)
                        oe.append(o)

            # ---------------- tail B: logitsT = fc_w @ oe.T + b ----------------
            logT = pp.tile([C, NB], f32, tag="logT")
            with tc.tile_pool(name="ps3", bufs=2, space="PSUM") as ps3:
                for it in range(NIT):
                    oT = []
                    for hh in range(H // P):
                        tps = ps3.tile([P, P], f32, tag="tps")
                        nc.tensor.transpose(tps[:], oe[it][:, hh * P:(hh + 1) * P],
                                            ident[:])
                        ot = tp.tile([P, P], f32, tag="ot")
                        nc.vector.tensor_copy(ot[:], tps[:])
                        oT.append(ot)
                    lps = ps3.tile([C, P], f32, tag="lps")
                    for hh in range(H // P):
                        nc.tensor.matmul(lps[:], fcw_sb[hh][:], oT[hh][:],
                                         start=(hh == 0), stop=(hh == H // P - 1))
                    nc.vector.tensor_scalar(out=logT[:, it * P:(it + 1) * P],
                                            in0=lps[:], scalar1=fcb_sb[:],
                                            scalar2=None, op0=OP.add)
            nc.sync.dma_start(logitsT[:], logT[:])

    nc.compile()
    return nc


# ---------------------------------------------------------------------------
# Host-side prep + execution
# ---------------------------------------------------------------------------

_CACHE = {}


def _get_nc(cfg: GatConfig):
    k = cfg.key()
    if k not in _CACHE:
        _CACHE[k] = build_gat(cfg)
    return _CACHE[k]


def prep_inputs(cfg, x, edge_index, W, a1, a2, fc_w, fc_b):
    """Shard + pack host inputs -> list of per-core in_maps."""
    bf = ml_dtypes.bfloat16
    N, NB = cfg.n, cfg.nb
    x = np.asarray(x, np.float32)
    W = np.asarray(W, np.float32)
    xT = np.ascontiguousarray(x.T).astype(bf)                    # [D, N]
    b1 = (W.T @ np.asarray(a1, np.float32)).astype(np.float32)   # [D, 1]
    b2 = (W.T @ np.asarray(a2, np.float32)).astype(np.float32)
    wcat = np.concatenate([W.T, b2], axis=1).astype(bf)          # [D, H+1]
    b1rep = np.repeat(b1, P, axis=1).astype(bf)                  # [D, P]
    fcwT = np.ascontiguousarray(np.asarray(fc_w, np.float32).T)  # [H, C]
    fcb = np.asarray(fc_b, np.float32).reshape(-1, 1)            # [C, 1]

    src = np.asarray(edge_index[0])
    dst = np.asarray(edge_index[1])
    in_maps = []
    for c in range(cfg.n_cores):
        lo = c * NB
        maskT = np.zeros((N, NB), np.float32)
        sel = (src >= lo) & (src < lo + NB)
        maskT[dst[sel], src[sel] - lo] = 1.0
        diag = np.arange(NB)
        maskT[lo + diag, diag] = 1.0
        in_maps.append({
            "xT": xT,
            "xTown": np.ascontiguousarray(xT[:, lo:lo + NB]),
            "wcat": wcat,
            "b1rep": b1rep,
            "maskT": maskT.astype(bf),
            "fcwT": fcwT,
            "fcb": fcb,
        })
    return in_maps


def run(cfg, inputs, trace=False):
    """Compile (cached), run on the 8 cores, return (logits, BassKernelResults)."""
    _install_ntff_hook()
    from concourse.bass_utils import run_bass_kernel_spmd

    nc = _get_nc(cfg)
    in_maps = prep_inputs(cfg, **inputs)
    res = run_bass_kernel_spmd(nc, in_maps, core_ids=list(range(cfg.n_cores)),
                               trace=trace)
    logits = np.concatenate(
        [np.asarray(res.results[c]["logitsT"], np.float32).T
         for c in range(cfg.n_cores)], axis=0)
    return logits, res


def kernel(x, edge_index, W, a1, a2, fc_w, fc_b):
    cfg = GatConfig(n=x.shape[0], d=x.shape[1], h=W.shape[0], c=fc_w.shape[0])
    logits, _ = run(cfg, dict(x=x, edge_index=edge_index, W=W, a1=a1, a2=a2,
                              fc_w=fc_w, fc_b=fc_b))
    return logits


# revision 25
# speedup vs baseline: 1.1339x; 1.1055x over previous
"""GAT (dense masked softmax attention) Bass kernel for 8 Trainium2 NeuronCores.

Row-parallel sharding: core c owns output rows [c*NB, (c+1)*NB). Each core
computes the full h = x @ W.T (replicated), then its row-block of the masked
attention softmax against all N nodes, in transposed layout (j on partitions,
own-rows on free dim), accumulating z @ [h | 1] in PSUM so the softmax
denominator falls out of the same matmuls.

Host prep (sharding only): x -> x.T, adjacency row-block -> additive mask
(0 at edges / -1e4 elsewhere) transposed to [N, NB], weight packing
(Wcat = [W.T | W.T@a2], b1 = W.T@a1 replicated across 128 columns).
"""

import contextlib
import ctypes
import sys
import types

import numpy as np
import ml_dtypes

import concourse.bacc as bacc
import concourse.mybir as mybir
import concourse.tile as tile
from concourse.masks import make_identity

P = 128
NEG_MASK = -1.0e4  # additive mask; prelu scales it by alpha=0.01 -> exp(~-100) ~ 0


def _install_ntff_hook():
    """Register the axon NTFF profile hook so run_bass_kernel_spmd(trace=True)
    can capture neuron-profile data (antenv.axon_hooks is absent here)."""
    if "antenv.axon_hooks" in sys.modules:
        return
    try:
        lib = ctypes.CDLL("/opt/axon/libaxon_pjrt.so")
        if not hasattr(lib, "axon_start_nrt_profile"):
            return
    except OSError:
        return
    lib.axon_start_nrt_profile.argtypes = [ctypes.POINTER(ctypes.c_int64), ctypes.c_size_t]
    lib.axon_start_nrt_profile.restype = ctypes.c_int64
    lib.axon_stop_nrt_profile.argtypes = [ctypes.c_char_p]
    lib.axon_stop_nrt_profile.restype = ctypes.c_int64

    @contextlib.contextmanager
    def _hook(output_dir, device_ids):
        import jax

        jax.devices()
        if device_ids:
            ids = (ctypes.c_int64 * len(device_ids))(*device_ids)
            rc = lib.axon_start_nrt_profile(ids, len(device_ids))
        else:
            rc = lib.axon_start_nrt_profile(None, 0)
        if rc != 0:
            raise RuntimeError(f"axon_start_nrt_profile rc={rc}")
        try:
            yield
        finally:
            n = lib.axon_stop_nrt_profile(str(output_dir).encode())
            print(f"ntff profile: {n} file(s) in {output_dir}", file=sys.stderr)

    mod = types.ModuleType("antenv.axon_hooks")
    mod.get_axon_ntff_profile_hook = lambda: _hook
    mod.set_axon_ntff_profile_hook = lambda h: None
    sys.modules["antenv.axon_hooks"] = mod


class GatConfig:
    def __init__(self, n=8192, d=512, h=256, c=16, n_cores=8, s_f32=True):
        assert n % (n_cores * P) == 0 and d % P == 0 and h % P == 0
        self.n, self.d, self.h, self.c, self.n_cores = n, d, h, c, n_cores
        self.nb = n // n_cores          # own rows per core
        self.nch = n // P               # j-chunks (also m-tiles of h)
        self.ndc = d // P               # feature chunks
        self.nit = self.nb // P         # own i-tiles
        self.dt_x = mybir.dt.bfloat16   # x / weights path
        self.dt_z = mybir.dt.bfloat16   # post-exp z and h matmul operands
        self.dt_s = mybir.dt.float32 if s_f32 else mybir.dt.bfloat16  # pre-exp path

    def key(self):
        return (self.n, self.d, self.h, self.c, self.n_cores, self.dt_s)


def build_gat(cfg: GatConfig):
    """Build + compile the SPMD Bass program (identical on all cores)."""
    nc = bacc.Bacc("TRN2", target_bir_lowering=False, debug=False,
                   enable_asserts=False, num_devices=cfg.n_cores)
    N, D, H, C = cfg.n, cfg.d, cfg.h, cfg.c
    NB, NCH, NDC, NIT = cfg.nb, cfg.nch, cfg.ndc, cfg.nit
    f32 = mybir.dt.float32
    bf16 = mybir.dt.bfloat16

    xT = nc.dram_tensor("xT", [D, N], cfg.dt_x, kind="ExternalInput").ap()
    xTown = nc.dram_tensor("xTown", [D, NB], cfg.dt_x, kind="ExternalInput").ap()
    wcat = nc.dram_tensor("wcat", [D, H + 1], cfg.dt_x, kind="ExternalInput").ap()
    b1rep = nc.dram_tensor("b1rep", [D, P], cfg.dt_x, kind="ExternalInput").ap()
    maskT = nc.dram_tensor("maskT", [N, NB], bf16, kind="ExternalInput").ap()
    fcwT = nc.dram_tensor("fcwT", [H, C], f32, kind="ExternalInput").ap()
    fcb = nc.dram_tensor("fcb", [C, 1], f32, kind="ExternalInput").ap()
    logitsT = nc.dram_tensor("logitsT", [C, NB], f32, kind="ExternalOutput").ap()

    AF = mybir.ActivationFunctionType
    OP = mybir.AluOpType

    with tile.TileContext(nc) as tc:
        with (
            tc.tile_pool(name="persist", bufs=1) as pp,
            tc.tile_pool(name="mwork", bufs=6) as mwp,
            tc.tile_pool(name="swork", bufs=3) as swp,
            tc.tile_pool(name="zwork", bufs=3) as zwp,
            tc.tile_pool(name="tail", bufs=2) as tp,
        ):
            # ---------------- resident inputs ----------------
            xo_sb = []
            for dd in range(NDC):
                t = pp.tile([P, NB], cfg.dt_x, tag=f"xo{dd}")
                nc.sync.dma_start(t[:], xTown[dd * P:(dd + 1) * P, :])
                xo_sb.append(t)
            wcat_sb = []
            for dd in range(NDC):
                t = pp.tile([P, H + 1], cfg.dt_x, tag=f"wc{dd}")
                nc.sync.dma_start(t[:], wcat[dd * P:(dd + 1) * P, :])
                wcat_sb.append(t)
            b1_sb = []
            for dd in range(NDC):
                t = pp.tile([P, P], cfg.dt_x, tag=f"b1{dd}")
                nc.sync.dma_start(t[:], b1rep[dd * P:(dd + 1) * P, :])
                b1_sb.append(t)
            fcw_sb = []
            for hh in range(H // P):
                t = pp.tile([P, C], f32, tag=f"fcw{hh}")
                nc.sync.dma_start(t[:], fcwT[hh * P:(hh + 1) * P, :])
                fcw_sb.append(t)
            fcb_sb = pp.tile([C, 1], f32, tag="fcb")
            nc.sync.dma_start(fcb_sb[:], fcb[:])
            ident = pp.tile([P, P], f32, tag="ident")
            make_identity(nc, ident[:])

            f1b_sb = pp.tile([P, NB], cfg.dt_s, tag="f1b")
            f2_sb = pp.tile([P, NCH], f32, tag="f2")
            h_sb = [pp.tile([P, H], cfg.dt_z, tag=f"h{m}", name=f"h{m}")
                    for m in range(NCH)]
            onecol = pp.tile([P, 1], cfg.dt_z, tag="onecol")
            nc.gpsimd.memset(onecol[:], 1.0)
            one1 = pp.tile([1, 1], f32, tag="one1")
            nc.gpsimd.memset(one1[:], 1.0)

            MB = NIT                     # m-tiles per xT column block
            GROUP = 2 if NCH % 2 == 0 else 1  # chunks per wide ACT op
            NACC = (NIT + 1) // 2        # packed accumulator tiles (2 i-tiles/bank)
            nq = (NB + 511) // 512       # 512-wide column groups of NB
            xtb = {}

            # Accumulators (4 banks) + denominator rows (2 banks) coexist with
            # the h-pipeline psum (2 banks): merged single-pass emission, so
            # the first sm/ACT work is not queued behind all of phase 1 on the
            # in-order DVE/sync engines.
            with tc.tile_pool(name="acc", bufs=1, space="PSUM") as accp:
                acc = [accp.tile([P, 2 * H], f32, tag=f"acc{q}", name=f"acc{q}")
                       for q in range(NACC)]
                dn = [accp.tile([1, min(512, NB - q * 512)], f32, tag=f"dn{q}",
                                name=f"dn{q}")
                      for q in range(nq)]

                with tc.tile_pool(name="ps1", bufs=2, space="PSUM") as ps1:
                    # f1b[p, i] = sum_d b1rep[d, p] * xTown[d, i] (same for all
                    # p); borrows acc tiles as transient psum before the
                    # accumulation groups start.
                    for q in range(nq):
                        w = min(512, NB - q * 512)
                        fps = acc[q % NACC][:, 0:w]
                        for dd in range(NDC):
                            nc.tensor.matmul(fps, b1_sb[dd][:],
                                             xo_sb[dd][:, q * 512:q * 512 + w],
                                             start=(dd == 0), stop=(dd == NDC - 1))
                        nc.vector.tensor_copy(f1b_sb[:, q * 512:q * 512 + w], fps)

                    # Per-chunk front end, multiplicative mask applied
                    # AFTER exp: ACT's Prelu reads the constant f1 broadcast
                    # tile directly (bias = f2 column), so ACT is gated only
                    # by a tiny f2 copy -- not by mask DMA or DVE grid ops.
                    for ch in range(NCH):
                        cb, mi = divmod(ch, MB)
                        if mi == 0:
                            for dd in range(NDC):
                                t = mwp.tile([P, MB * P], cfg.dt_x,
                                             tag=f"xtb{dd}", bufs=2,
                                             name=f"xtb{dd}_{cb}")
                                nc.sync.dma_start(
                                    t[:], xT[dd * P:(dd + 1) * P,
                                             cb * MB * P:(cb + 1) * MB * P])
                                xtb[dd, cb] = t
                        mk = mwp.tile([P, NB], bf16, tag="mask")
                        nc.sync.dma_start(mk[:], maskT[ch * P:(ch + 1) * P, :])

                        # h m-tile ch: [P, H+1] = sum_d xT_d[:, ch].T @ wcat_d
                        hps = ps1.tile([P, H + 1], f32, tag="hps")
                        for dd in range(NDC):
                            nc.tensor.matmul(hps[:],
                                             xtb[dd, cb][:, mi * P:(mi + 1) * P],
                                             wcat_sb[dd][:],
                                             start=(dd == 0), stop=(dd == NDC - 1))

                        u = swp.tile([P, NB], cfg.dt_s, tag="u")
                        if (ch % 3 == 1) and NCH >= 32:
                            # leaky-relu on DVE to relieve the scalar engine:
                            # u = max(s, 0.01*s), s = f1 + f2
                            sm = swp.tile([P, NB], cfg.dt_s, tag="sm", bufs=2)
                            nc.vector.tensor_scalar(out=sm[:], in0=f1b_sb[:],
                                                    scalar1=hps[:, H:H + 1],
                                                    scalar2=None, op0=OP.add)
                            p01 = swp.tile([P, NB], cfg.dt_s, tag="p01", bufs=2)
                            nc.vector.tensor_scalar_mul(p01[:], sm[:], 0.01)
                            nc.vector.tensor_tensor(out=u[:], in0=sm[:],
                                                    in1=p01[:], op=OP.max)
                        else:
                            f2c = tp.tile([P, 1], f32, tag="f2c", bufs=4)
                            nc.vector.tensor_copy(f2c[:], hps[:, H:H + 1])
                            nc.scalar.activation(u[:], f1b_sb[:], AF.Prelu,
                                                 bias=f2c[:], scale=1.0,
                                                 alpha=0.01)
                        ez = swp.tile([P, NB], cfg.dt_s, tag="ez")
                        nc.scalar.activation(ez[:], u[:], AF.Exp)
                        z = zwp.tile([P, NB], cfg.dt_z, tag="z", bufs=4)
                        nc.vector.tensor_tensor(out=z[:], in0=ez[:],
                                                in1=mk[:], op=OP.mult)
                        nc.vector.tensor_copy(h_sb[ch][:], hps[:, 0:H])

                        # inline accumulation: acc += z_it.T @ h ; dn += 1.T @ z
                        for it in range(NIT):
                            # Two accumulation groups share each psum bank;
                            # start=True clears the WHOLE bank, so only the
                            # even group (emitted first at ch==0) may clear.
                            # The odd group's first matmul writes (not
                            # accumulates): has_written is 0 there.
                            nc.tensor.matmul(
                                acc[it // 2][:, (it % 2) * H:(it % 2) * H + H],
                                z[:, it * P:(it + 1) * P], h_sb[ch][:],
                                start=(ch == 0 and it % 2 == 0),
                                stop=(ch == NCH - 1),
                                skip_group_check=True)
                        for q in range(nq):
                            w = min(512, NB - q * 512)
                            nc.tensor.matmul(dn[q][:], onecol[:],
                                             z[:, q * 512:q * 512 + w],
                                             start=(ch == 0),
                                             stop=(ch == NCH - 1))

                # ---------------- tail A: normalize + ELU ----------------
                dnrow = pp.tile([1, NB], f32, tag="dnrow")
                for q in range(nq):
                    w = min(512, NB - q * 512)
                    nc.vector.tensor_copy(dnrow[0:1, q * 512:q * 512 + w], dn[q][:])
                oe = []
                with tc.tile_pool(name="psD", bufs=2, space="PSUM") as psD:
                    for it in range(NIT):
                        dnc = psD.tile([P, 1], f32, tag="dnc")
                        nc.tensor.matmul(dnc[:],
                                         dnrow[0:1, it * P:(it + 1) * P],
                                         one1[:], start=True, stop=True)
                        rec = tp.tile([P, 1], f32, tag="rec")
                        nc.vector.reciprocal(rec[:], dnc[:])
                        recn = tp.tile([P, 1], f32, tag="recn")
                        nc.vector.tensor_scalar_mul(recn[:], rec[:], -1.0)
                        a = acc[it // 2][:, (it % 2) * H:(it % 2) * H + H]
                        # elu(on) = relu(on) + exp(min(on, 0)) - 1, with
                        # on = acc/denom done via per-partition scale APs
                        pos = tp.tile([P, H], f32, tag="pos")
                        nc.scalar.activation(pos[:], a, AF.Relu, scale=rec[:])
                        r2 = tp.tile([P, H], f32, tag="r2")
                        nc.scalar.activation(r2[:], a, AF.Relu, scale=recn[:])
                        ex = tp.tile([P, H], f32, tag="ex")
                        nc.scalar.activation(ex[:], r2[:], AF.Exp, scale=-1.0)
                        o = pp.tile([P, H], f32, tag=f"oe{it}", name=f"oe{it}")
                        nc.vector.scalar_tensor_tensor(out=o[:], in0=ex[:],
                                                       scalar=-1.0, in1=pos[:],
                                                       op0=OP.add, op1=OP.add)
                        oe.append(o)

            # ---------------- tail B: logitsT = fc_w @ oe.T + b ----------------
            logT = pp.tile([C, NB], f32, tag="logT")
            with tc.tile_pool(name="ps3", bufs=2, space="PSUM") as ps3:
                for it in range(NIT):
                    oT = []
                    for hh in range(H // P):
                        tps = ps3.tile([P, P], f32, tag="tps")
                        nc.tensor.transpose(tps[:], oe[it][:, hh * P:(hh + 1) * P],
                                            ident[:])
                        ot = tp.tile([P, P], f32, tag="ot")
                        nc.vector.tensor_copy(ot[:], tps[:])
                        oT.append(ot)
                    lps = ps3.tile([C, P], f32, tag="lps")
                    for hh in range(H // P):
                        nc.tensor.matmul(lps[:], fcw_sb[hh][:], oT[hh][:],
                                         start=(hh == 0), stop=(hh == H // P - 1))
                    nc.vector.tensor_scalar(out=logT[:, it * P:(it + 1) * P],
                                            in0=lps[:], scalar1=fcb_sb[:],
                                            scalar2=None, op0=OP.add)
            nc.sync.dma_start(logitsT[:], logT[:])

    nc.compile()
    return nc


# ---------------------------------------------------------------------------
# Host-side prep + execution
# ---------------------------------------------------------------------------

_CACHE = {}


def _get_nc(cfg: GatConfig):
    k = cfg.key()
    if k not in _CACHE:
        _CACHE[k] = build_gat(cfg)
    return _CACHE[k]


def prep_inputs(cfg, x, edge_index, W, a1, a2, fc_w, fc_b):
    """Shard + pack host inputs -> list of per-core in_maps."""
    bf = ml_dtypes.bfloat16
    N, NB = cfg.n, cfg.nb
    x = np.asarray(x, np.float32)
    W = np.asarray(W, np.float32)
    xT = np.ascontiguousarray(x.T).astype(bf)                    # [D, N]
    b1 = (W.T @ np.asarray(a1, np.float32)).astype(np.float32)   # [D, 1]
    b2 = (W.T @ np.asarray(a2, np.float32)).astype(np.float32)
    wcat = np.concatenate([W.T, b2], axis=1).astype(bf)          # [D, H+1]
    b1rep = np.repeat(b1, P, axis=1).astype(bf)                  # [D, P]
    fcwT = np.ascontiguousarray(np.asarray(fc_w, np.float32).T)  # [H, C]
    fcb = np.asarray(fc_b, np.float32).reshape(-1, 1)            # [C, 1]

    src = np.asarray(edge_index[0])
    dst = np.asarray(edge_index[1])
    in_maps = []
    for c in range(cfg.n_cores):
        lo = c * NB
        maskT = np.zeros((N, NB), np.float32)
        sel = (src >= lo) & (src < lo + NB)
        maskT[dst[sel], src[sel] - lo] = 1.0
        diag = np.arange(NB)
        maskT[lo + diag, diag] = 1.0
        in_maps.append({
            "xT": xT,
            "xTown": np.ascontiguousarray(xT[:, lo:lo + NB]),
            "wcat": wcat,
            "b1rep": b1rep,
            "maskT": maskT.astype(bf),
            "fcwT": fcwT,
            "fcb": fcb,
        })
    return in_maps


def run(cfg, inputs, trace=False):
    """Compile (cached), run on the 8 cores, return (logits, BassKernelResults)."""
    _install_ntff_hook()
    from concourse.bass_utils import run_bass_kernel_spmd

    nc = _get_nc(cfg)
    in_maps = prep_inputs(cfg, **inputs)
    res = run_bass_kernel_spmd(nc, in_maps, core_ids=list(range(cfg.n_cores)),
                               trace=trace)
    logits = np.concatenate(
        [np.asarray(res.results[c]["logitsT"], np.float32).T
         for c in range(cfg.n_cores)], axis=0)
    return logits, res


def kernel(x, edge_index, W, a1, a2, fc_w, fc_b):
    cfg = GatConfig(n=x.shape[0], d=x.shape[1], h=W.shape[0], c=fc_w.shape[0])
    logits, _ = run(cfg, dict(x=x, edge_index=edge_index, W=W, a1=a1, a2=a2,
                              fc_w=fc_w, fc_b=fc_b))
    return logits


# revision 28
# speedup vs baseline: 1.1658x; 1.0282x over previous
"""GAT (dense masked softmax attention) Bass kernel for 8 Trainium2 NeuronCores.

Row-parallel sharding: core c owns output rows [c*NB, (c+1)*NB). Each core
computes the full h = x @ W.T (replicated), then its row-block of the masked
attention softmax against all N nodes, in transposed layout (j on partitions,
own-rows on free dim), accumulating z @ [h | 1] in PSUM so the softmax
denominator falls out of the same matmuls.

Host prep (sharding only): x -> x.T, adjacency row-block -> additive mask
(0 at edges / -1e4 elsewhere) transposed to [N, NB], weight packing
(Wcat = [W.T | W.T@a2], b1 = W.T@a1 replicated across 128 columns).
"""

import contextlib
import ctypes
import sys
import types

import numpy as np
import ml_dtypes

import concourse.bacc as bacc
import concourse.mybir as mybir
import concourse.tile as tile

P = 128
NEG_MASK = -1.0e4  # additive mask; prelu scales it by alpha=0.01 -> exp(~-100) ~ 0


def _install_ntff_hook():
    """Register the axon NTFF profile hook so run_bass_kernel_spmd(trace=True)
    can capture neuron-profile data (antenv.axon_hooks is absent here)."""
    if "antenv.axon_hooks" in sys.modules:
        return
    try:
        lib = ctypes.CDLL("/opt/axon/libaxon_pjrt.so")
        if not hasattr(lib, "axon_start_nrt_profile"):
            return
    except OSError:
        return
    lib.axon_start_nrt_profile.argtypes = [ctypes.POINTER(ctypes.c_int64), ctypes.c_size_t]
    lib.axon_start_nrt_profile.restype = ctypes.c_int64
    lib.axon_stop_nrt_profile.argtypes = [ctypes.c_char_p]
    lib.axon_stop_nrt_profile.restype = ctypes.c_int64

    @contextlib.contextmanager
    def _hook(output_dir, device_ids):
        import jax

        jax.devices()
        if device_ids:
            ids = (ctypes.c_int64 * len(device_ids))(*device_ids)
            rc = lib.axon_start_nrt_profile(ids, len(device_ids))
        else:
            rc = lib.axon_start_nrt_profile(None, 0)
        if rc != 0:
            raise RuntimeError(f"axon_start_nrt_profile rc={rc}")
        try:
            yield
        finally:
            n = lib.axon_stop_nrt_profile(str(output_dir).encode())
            print(f"ntff profile: {n} file(s) in {output_dir}", file=sys.stderr)

    mod = types.ModuleType("antenv.axon_hooks")
    mod.get_axon_ntff_profile_hook = lambda: _hook
    mod.set_axon_ntff_profile_hook = lambda h: None
    sys.modules["antenv.axon_hooks"] = mod


class GatConfig:
    def __init__(self, n=8192, d=512, h=256, c=16, n_cores=8, s_f32=True):
        assert n % (n_cores * P) == 0 and d % P == 0 and h % P == 0
        self.n, self.d, self.h, self.c, self.n_cores = n, d, h, c, n_cores
        self.nb = n // n_cores          # own rows per core
        self.nch = n // P               # j-chunks (also m-tiles of h)
        self.ndc = d // P               # feature chunks
        self.nit = self.nb // P         # own i-tiles
        self.dt_x = mybir.dt.bfloat16   # x / weights path
        self.dt_z = mybir.dt.bfloat16   # post-exp z and h matmul operands
        self.dt_s = mybir.dt.float32 if s_f32 else mybir.dt.bfloat16  # pre-exp path

    def key(self):
        return (self.n, self.d, self.h, self.c, self.n_cores, self.dt_s)


def build_gat(cfg: GatConfig):
    """Build + compile the SPMD Bass program (identical on all cores)."""
    nc = bacc.Bacc("TRN2", target_bir_lowering=False, debug=False,
                   enable_asserts=False, num_devices=cfg.n_cores)
    N, D, H, C = cfg.n, cfg.d, cfg.h, cfg.c
    NB, NCH, NDC, NIT = cfg.nb, cfg.nch, cfg.ndc, cfg.nit
    f32 = mybir.dt.float32
    bf16 = mybir.dt.bfloat16

    xT = nc.dram_tensor("xT", [D, N], cfg.dt_x, kind="ExternalInput").ap()
    xTown = nc.dram_tensor("xTown", [D, NB], cfg.dt_x, kind="ExternalInput").ap()
    wcat = nc.dram_tensor("wcat", [D, H + 1], cfg.dt_x, kind="ExternalInput").ap()
    b1rep = nc.dram_tensor("b1rep", [D, P], cfg.dt_x, kind="ExternalInput").ap()
    maskT = nc.dram_tensor("maskT", [N, NB], bf16, kind="ExternalInput").ap()
    fcwT = nc.dram_tensor("fcwT", [H, C], f32, kind="ExternalInput").ap()
    fcb = nc.dram_tensor("fcb", [C, 1], f32, kind="ExternalInput").ap()
    logitsT = nc.dram_tensor("logitsT", [C, NB], f32, kind="ExternalOutput").ap()

    AF = mybir.ActivationFunctionType
    OP = mybir.AluOpType

    with tile.TileContext(nc) as tc:
        with (
            tc.tile_pool(name="persist", bufs=1) as pp,
            tc.tile_pool(name="mwork", bufs=6) as mwp,
            tc.tile_pool(name="swork", bufs=3) as swp,
            tc.tile_pool(name="zwork", bufs=3) as zwp,
            tc.tile_pool(name="tail", bufs=2) as tp,
        ):
            # ---------------- resident inputs ----------------
            xo_sb = []
            for dd in range(NDC):
                t = pp.tile([P, NB], cfg.dt_x, tag=f"xo{dd}")
                nc.sync.dma_start(t[:], xTown[dd * P:(dd + 1) * P, :])
                xo_sb.append(t)
            wcat_sb = []
            for dd in range(NDC):
                t = pp.tile([P, H + 1], cfg.dt_x, tag=f"wc{dd}")
                nc.sync.dma_start(t[:], wcat[dd * P:(dd + 1) * P, :])
                wcat_sb.append(t)
            b1_sb = []
            for dd in range(NDC):
                t = pp.tile([P, P], cfg.dt_x, tag=f"b1{dd}")
                nc.sync.dma_start(t[:], b1rep[dd * P:(dd + 1) * P, :])
                b1_sb.append(t)
            fcw_sb = []
            for hh in range(H // P):
                t = pp.tile([P, C], f32, tag=f"fcw{hh}")
                nc.sync.dma_start(t[:], fcwT[hh * P:(hh + 1) * P, :])
                fcw_sb.append(t)
            fcb_sb = pp.tile([C, 1], f32, tag="fcb")
            nc.sync.dma_start(fcb_sb[:], fcb[:])

            f1b_sb = pp.tile([P, NB], cfg.dt_s, tag="f1b")
            h_sb = [pp.tile([P, H], cfg.dt_z, tag=f"h{m}", name=f"h{m}")
                    for m in range(NCH)]
            onecol = pp.tile([P, 1], cfg.dt_z, tag="onecol")
            nc.gpsimd.memset(onecol[:], 1.0)
            onerow = pp.tile([1, P], f32, tag="onerow")
            nc.gpsimd.memset(onerow[:], 1.0)

            MB = NIT                     # m-tiles per xT column block
            NH = H // P                  # stationary h halves
            nq = (NB + 511) // 512       # 512-wide column groups of NB
            xtb = {}

            # Transposed accumulators accT[half][q] [P, 512] (4 banks) +
            # denominator rows (nq banks) + h-pipeline psum (2 banks) = 8.
            # h is the STATIONARY matmul operand (2 LDWs/chunk, hidden under
            # N=512 streams); z feeds straight through as the moving operand.
            with tc.tile_pool(name="acc", bufs=1, space="PSUM") as accp:
                accT = [[accp.tile([P, min(512, NB - q * 512)], f32,
                                   tag=f"accT{hh}_{q}", name=f"accT{hh}_{q}")
                         for q in range(nq)] for hh in range(NH)]
                dn = [accp.tile([1, min(512, NB - q * 512)], f32, tag=f"dn{q}",
                                name=f"dn{q}")
                      for q in range(nq)]

                with tc.tile_pool(name="ps1", bufs=2, space="PSUM") as ps1:
                    # f1b[p, i] = sum_d b1rep[d, p] * xTown[d, i] (same value
                    # on every partition p)
                    for q in range(nq):
                        w = min(512, NB - q * 512)
                        fps = accT[0][q][:, 0:w]  # transient psum reuse
                        for dd in range(NDC):
                            nc.tensor.matmul(fps, b1_sb[dd][:],
                                             xo_sb[dd][:, q * 512:q * 512 + w],
                                             start=(dd == 0), stop=(dd == NDC - 1))
                        nc.vector.tensor_copy(f1b_sb[:, q * 512:q * 512 + w],
                                              fps)

                    # Per-chunk front end, multiplicative mask applied AFTER
                    # exp: ACT's Prelu reads the constant f1 broadcast tile
                    # (bias = f2 column), so ACT is gated only by a tiny f2
                    # copy -- not by mask DMA or DVE grid ops.
                    for ch in range(NCH):
                        cb, mi = divmod(ch, MB)
                        if mi == 0:
                            for dd in range(NDC):
                                t = mwp.tile([P, MB * P], cfg.dt_x,
                                             tag=f"xtb{dd}", bufs=2,
                                             name=f"xtb{dd}_{cb}")
                                nc.sync.dma_start(
                                    t[:], xT[dd * P:(dd + 1) * P,
                                             cb * MB * P:(cb + 1) * MB * P])
                                xtb[dd, cb] = t
                        mk = mwp.tile([P, NB], bf16, tag="mask")
                        nc.sync.dma_start(mk[:], maskT[ch * P:(ch + 1) * P, :])

                        # h m-tile ch: [P, H+1] = sum_d xT_d[:, ch].T @ wcat_d
                        hps = ps1.tile([P, H + 1], f32, tag="hps")
                        for dd in range(NDC):
                            nc.tensor.matmul(hps[:],
                                             xtb[dd, cb][:, mi * P:(mi + 1) * P],
                                             wcat_sb[dd][:],
                                             start=(dd == 0), stop=(dd == NDC - 1))

                        u = swp.tile([P, NB], cfg.dt_s, tag="u")
                        if (ch % 3 == 1) and NCH >= 32:
                            # leaky-relu on DVE to relieve the scalar engine:
                            # u = max(s, 0.01*s), s = f1 + f2
                            sm = swp.tile([P, NB], cfg.dt_s, tag="sm", bufs=2)
                            nc.vector.tensor_scalar(out=sm[:], in0=f1b_sb[:],
                                                    scalar1=hps[:, H:H + 1],
                                                    scalar2=None, op0=OP.add)
                            p01 = swp.tile([P, NB], cfg.dt_s, tag="p01", bufs=2)
                            nc.vector.tensor_scalar_mul(p01[:], sm[:], 0.01)
                            nc.vector.tensor_tensor(out=u[:], in0=sm[:],
                                                    in1=p01[:], op=OP.max)
                        else:
                            f2c = tp.tile([P, 1], f32, tag="f2c", bufs=4)
                            nc.vector.tensor_copy(f2c[:], hps[:, H:H + 1])
                            nc.scalar.activation(u[:], f1b_sb[:], AF.Prelu,
                                                 bias=f2c[:], scale=1.0,
                                                 alpha=0.01)
                        ez = swp.tile([P, NB], cfg.dt_s, tag="ez")
                        nc.scalar.activation(ez[:], u[:], AF.Exp)
                        z = zwp.tile([P, NB], cfg.dt_z, tag="z", bufs=4)
                        nc.vector.tensor_tensor(out=z[:], in0=ez[:],
                                                in1=mk[:], op=OP.mult)
                        nc.vector.tensor_copy(h_sb[ch][:], hps[:, 0:H])

                        # accT[half][q] += h_half.T @ z ; dn += 1.T @ z
                        for hh in range(NH):
                            for q in range(nq):
                                w = min(512, NB - q * 512)
                                nc.tensor.matmul(
                                    accT[hh][q][:],
                                    h_sb[ch][:, hh * P:(hh + 1) * P],
                                    z[:, q * 512:q * 512 + w],
                                    start=(ch == 0), stop=(ch == NCH - 1))
                        for q in range(nq):
                            w = min(512, NB - q * 512)
                            nc.tensor.matmul(dn[q][:], onecol[:],
                                             z[:, q * 512:q * 512 + w],
                                             start=(ch == 0),
                                             stop=(ch == NCH - 1))

                # ------------ tail A: normalize + ELU (transposed) ------------
                dnrow = pp.tile([1, NB], f32, tag="dnrow")
                for q in range(nq):
                    w = min(512, NB - q * 512)
                    nc.vector.tensor_copy(dnrow[0:1, q * 512:q * 512 + w], dn[q][:])
                recrow = pp.tile([1, NB], f32, tag="recrow")
                nc.vector.reciprocal(recrow[:], dnrow[:])
                oeT = []
                with tc.tile_pool(name="psR", bufs=2, space="PSUM") as psR:
                    rb_sb = []
                    for q in range(nq):
                        w = min(512, NB - q * 512)
                        rb = psR.tile([P, w], f32, tag="rb")
                        nc.tensor.matmul(rb[:], onerow[:],
                                         recrow[0:1, q * 512:q * 512 + w],
                                         start=True, stop=True)
                        rs = tp.tile([P, NB if False else 512], f32, tag="rs",
                                     bufs=2)
                        nc.vector.tensor_copy(rs[:, 0:w], rb[:])
                        rb_sb.append(rs)
                    for hh in range(NH):
                        row = []
                        for q in range(nq):
                            w = min(512, NB - q * 512)
                            on = tp.tile([P, 512], f32, tag="on", bufs=2)
                            nc.vector.tensor_tensor(out=on[:, 0:w],
                                                    in0=accT[hh][q][:],
                                                    in1=rb_sb[q][:, 0:w],
                                                    op=OP.mult)
                            pos = tp.tile([P, 512], f32, tag="pos", bufs=2)
                            nc.vector.tensor_scalar(out=pos[:, 0:w],
                                                    in0=on[:, 0:w], scalar1=0.0,
                                                    scalar2=None, op0=OP.max)
                            ngm = tp.tile([P, 512], f32, tag="ngm", bufs=2)
                            nc.vector.tensor_scalar(out=ngm[:, 0:w],
                                                    in0=on[:, 0:w], scalar1=0.0,
                                                    scalar2=None, op0=OP.min)
                            ex = tp.tile([P, 512], f32, tag="ex", bufs=2)
                            nc.scalar.activation(ex[:, 0:w], ngm[:, 0:w], AF.Exp)
                            o = pp.tile([P, 512], f32, tag=f"oeT{hh}_{q}",
                                        name=f"oeT{hh}_{q}")
                            nc.vector.scalar_tensor_tensor(out=o[:, 0:w],
                                                           in0=ex[:, 0:w],
                                                           scalar=-1.0,
                                                           in1=pos[:, 0:w],
                                                           op0=OP.add,
                                                           op1=OP.add)
                            row.append(o)
                        oeT.append(row)

            # -------- tail B: logitsT = fc_w @ oeT + b (no transposes) --------
            logT = pp.tile([C, NB], f32, tag="logT")
            with tc.tile_pool(name="ps3", bufs=2, space="PSUM") as ps3:
                for q in range(nq):
                    w = min(512, NB - q * 512)
                    lps = ps3.tile([C, 512], f32, tag="lps")
                    for hh in range(NH):
                        nc.tensor.matmul(lps[:, 0:w], fcw_sb[hh][:],
                                         oeT[hh][q][:, 0:w],
                                         start=(hh == 0), stop=(hh == NH - 1))
                    nc.vector.tensor_scalar(out=logT[:, q * 512:q * 512 + w],
                                            in0=lps[:, 0:w], scalar1=fcb_sb[:],
                                            scalar2=None, op0=OP.add)
            nc.sync.dma_start(logitsT[:], logT[:])

    nc.compile()
    return nc


# ---------------------------------------------------------------------------
# Host-side prep + execution
# ---------------------------------------------------------------------------

_CACHE = {}


def _get_nc(cfg: GatConfig):
    k = cfg.key()
    if k not in _CACHE:
        _CACHE[k] = build_gat(cfg)
    return _CACHE[k]


def prep_inputs(cfg, x, edge_index, W, a1, a2, fc_w, fc_b):
    """Shard + pack host inputs -> list of per-core in_maps."""
    bf = ml_dtypes.bfloat16
    N, NB = cfg.n, cfg.nb
    x = np.asarray(x, np.float32)
    W = np.asarray(W, np.float32)
    xT = np.ascontiguousarray(x.T).astype(bf)                    # [D, N]
    b1 = (W.T @ np.asarray(a1, np.float32)).astype(np.float32)   # [D, 1]
    b2 = (W.T @ np.asarray(a2, np.float32)).astype(np.float32)
    wcat = np.concatenate([W.T, b2], axis=1).astype(bf)          # [D, H+1]
    b1rep = np.repeat(b1, P, axis=1).astype(bf)                  # [D, P]
    fcwT = np.ascontiguousarray(np.asarray(fc_w, np.float32).T)  # [H, C]
    fcb = np.asarray(fc_b, np.float32).reshape(-1, 1)            # [C, 1]

    src = np.asarray(edge_index[0])
    dst = np.asarray(edge_index[1])
    in_maps = []
    for c in range(cfg.n_cores):
        lo = c * NB
        maskT = np.zeros((N, NB), np.float32)
        sel = (src >= lo) & (src < lo + NB)
        maskT[dst[sel], src[sel] - lo] = 1.0
        diag = np.arange(NB)
        maskT[lo + diag, diag] = 1.0
        in_maps.append({
            "xT": xT,
            "xTown": np.ascontiguousarray(xT[:, lo:lo + NB]),
            "wcat": wcat,
            "b1rep": b1rep,
            "maskT": maskT.astype(bf),
            "fcwT": fcwT,
            "fcb": fcb,
        })
    return in_maps


def run(cfg, inputs, trace=False):
    """Compile (cached), run on the 8 cores, return (logits, BassKernelResults)."""
    _install_ntff_hook()
    from concourse.bass_utils import run_bass_kernel_spmd

    nc = _get_nc(cfg)
    in_maps = prep_inputs(cfg, **inputs)
    res = run_bass_kernel_spmd(nc, in_maps, core_ids=list(range(cfg.n_cores)),
                               trace=trace)
    logits = np.concatenate(
        [np.asarray(res.results[c]["logitsT"], np.float32).T
         for c in range(cfg.n_cores)], axis=0)
    return logits, res


def kernel(x, edge_index, W, a1, a2, fc_w, fc_b):
    cfg = GatConfig(n=x.shape[0], d=x.shape[1], h=W.shape[0], c=fc_w.shape[0])
    logits, _ = run(cfg, dict(x=x, edge_index=edge_index, W=W, a1=a1, a2=a2,
                              fc_w=fc_w, fc_b=fc_b))
    return logits
